# revision 22
# baseline (speedup 1.0000x reference)
import numpy as np

# nn_GaussianMixture: log-likelihood of N points under an M-component GMM.
# Shapes hardcoded per contract: points [500000,16], centers [128,16],
# covs_inv_sqrt [128,16,16], weights [128], threshold [1].
#
# Device strategy (8 NeuronCores, data-parallel over points):
#   d_ij = -0.5 (x_i-c_j)^T S_j (x_i-c_j) + logcoef_j, out_i = logsumexp_j d_ij
# Rewritten in centered coords y = x - 0.5 as
#   d_ij = y^T T_j y + l_j^T y + k_j
# A per-point affine shift s(y) = alpha*|y|^2 + beta^T y + gamma (fit at
# runtime to max_j d) is folded into the weights so a single exp pass is
# numerically safe:  d'_ij = d_ij - s(y_i),  out = s + log sum_j exp(d').
# The quadratic y^T (T_j - alpha I) y is evaluated through a rank-1 frame:
#   (T_j - alpha I) ~= sum_r G_jr a_r a_r^T  =>  y^T T'' y = sum_r G_jr (a_r.y)^2
# so the device pipeline per point-tile is pure matmul + square + matmul:
#   stage1 (PE):  v_r = a_r . y              [K=32 matmul on y_hi/y_lo bf16]
#   square (ACT/DVE): w_r = v_r^2            -> bf16
#   main   (PE):  d' = G^T w + L^T y + bias  [K=128 + K=32, accumulated in PSUM]
#   exp    (ACT): u = exp(d' + bias_j)       [PSUM -> SBUF bf16]
#   zsum   (PE):  z_i = sum_j u_ij           [ones matmul]
# Host: out = s + ln z - threshold; rare out-of-range points recomputed exactly.

N, M, D = 500000, 128, 16
N_CORES = 8
HALF = 512                      # half-span (one PSUM bank of fp32)
SPAN = 2 * HALF
NPC = (N + N_CORES - 1) // N_CORES          # 62500 points per core
NSPANS = (NPC + SPAN - 1) // SPAN           # 62
NH = 2 * NSPANS                              # 124 half-spans
NPAD = NSPANS * SPAN                         # 63488

_CACHE = {}


def _bf16(a):
    import ml_dtypes
    return np.asarray(a, np.float32).astype(ml_dtypes.bfloat16)


def _prep_model(centers, covs_inv_sqrt, weights):
    """Exact f64 model constants in centered coordinates y = x - 0.5."""
    c = np.asarray(centers, np.float64)
    Lm = np.asarray(covs_inv_sqrt, np.float64)
    S = np.einsum('jde,jfe->jdf', Lm, Lm)                    # [M,D,D]
    w = np.abs(np.asarray(weights, np.float64))
    cprs = w / (w.sum() + 1e-30)
    sign, logdet = np.linalg.slogdet(S)
    lc = np.log(cprs + 1e-300) + 0.5 * logdet                # [M]
    ct = c - 0.5                                             # centered centers
    T = -0.5 * S                                             # quad coeff
    lvec = np.einsum('jde,je->jd', S, ct)                    # [M,D]
    kconst = -0.5 * np.einsum('jd,jd->j', ct, lvec * 0 + np.einsum('jde,je->jd', S, ct)) \
        if False else -0.5 * np.einsum('jd,jde,je->j', ct, S, ct)
    kconst = kconst + lc                                     # [M]
    return S, T, lvec, kconst


def _d_exact(y, T, lvec, kconst):
    """Exact d matrix for a batch of centered points y [n,D] (f64)."""
    q = np.einsum('nd,jde,ne->nj', y, T, y, optimize=True)
    return q + y @ lvec.T + kconst[None, :]


def _fit_shift(y, T, lvec, kconst, rng):
    """Affine s(y)=alpha*|y|^2+beta.y+gamma with max_j d - s centered in a
    safe exp window."""
    n = y.shape[0]
    idx = rng.choice(n, size=min(3072, n), replace=False)
    ys = y[idx].astype(np.float64)
    dm = _d_exact(ys, T, lvec, kconst)
    mx = dm.max(1)
    F = np.concatenate([(ys * ys).sum(1, keepdims=True), ys,
                        np.ones((len(ys), 1))], axis=1)
    coef, *_ = np.linalg.lstsq(F, mx, rcond=None)
    r = mx - F @ coef
    coef[-1] += (r.min() + r.max()) / 2.0                    # center the window
    return coef                                              # [D+2]


def _build_frame(rng):
    """128 rank-1 directions: 16 coordinate axes + 112 random unit vectors,
    quantized to bf16 (the solve below uses the quantized values exactly)."""
    A = np.zeros((128, D))
    A[:16] = np.eye(D)
    V = rng.standard_normal((112, D))
    V /= np.linalg.norm(V, axis=1, keepdims=True)
    A[16:] = V
    return _bf16(A).astype(np.float64)


def _solve_frame_weights(A, Ttgt):
    """G [M,128] minimizing ||sum_r G_jr a_r a_r^T - Ttgt_j||_F."""
    iu0, iu1 = np.triu_indices(D)
    wgt = np.where(iu0 == iu1, 1.0, np.sqrt(2.0))
    B = (np.einsum('rd,re->rde', A, A)[:, iu0, iu1] * wgt)   # [128,136]
    Tv = Ttgt[:, iu0, iu1] * wgt                             # [M,136]
    G = Tv @ np.linalg.pinv(B)                               # [M,128]
    return G


def _build_bass():
    import concourse.bass as bass
    import concourse.mybir as mybir
    from concourse import tile
    from contextlib import ExitStack

    bf16 = mybir.dt.bfloat16
    f32 = mybir.dt.float32
    nc = bass.Bass()

    yt = nc.dram_tensor("yt", [32, NPAD], bf16, kind="ExternalInput")
    gt = nc.dram_tensor("gt", [128, 128], bf16, kind="ExternalInput")
    a2 = nc.dram_tensor("a2", [32, 128], bf16, kind="ExternalInput")
    c2w = nc.dram_tensor("c2w", [32, 128], bf16, kind="ExternalInput")
    bias = nc.dram_tensor("bias", [128, 1], f32, kind="ExternalInput")
    zout = nc.dram_tensor("zout", [(NH + 3) // 4, 128, HALF], f32,
                          kind="ExternalOutput")

    with tile.TileContext(nc) as tc, ExitStack() as ctx:
        const = ctx.enter_context(tc.tile_pool(name="const", bufs=1))
        phipool = ctx.enter_context(tc.tile_pool(name="phipool", bufs=4))
        upool = ctx.enter_context(tc.tile_pool(name="upool", bufs=3))
        lnzpool = ctx.enter_context(tc.tile_pool(name="lnzpool", bufs=2))
        vpool = ctx.enter_context(tc.tile_pool(name="vpool", bufs=2, space="PSUM"))
        dpool = ctx.enter_context(tc.tile_pool(name="dpool", bufs=2, space="PSUM"))
        zpool = ctx.enter_context(tc.tile_pool(name="zpool", bufs=2, space="PSUM"))

        gt_sb = const.tile([128, 128], bf16)
        a2_sb = const.tile([32, 128], bf16)
        c2w_sb = const.tile([32, 128], bf16)
        bias_sb = const.tile([128, 1], f32)
        ones_sb = const.tile([128, 1], bf16)
        nc.sync.dma_start(gt_sb[:], gt[:, :])
        nc.sync.dma_start(a2_sb[:], a2[:, :])
        nc.sync.dma_start(c2w_sb[:], c2w[:, :])
        nc.sync.dma_start(bias_sb[:], bias[:, :])
        nc.vector.memset(ones_sb[:], 1.0)

        # Preload all point data: one persistent SBUF region, chunked
        # first-write DMAs (no pool rotation -> no WAW sync waits).
        y_all = const.tile([32, NPAD], bf16)
        YCHUNK = 4 * HALF
        for cstart in range(0, NPAD, YCHUNK):
            nc.sync.dma_start(y_all[:, cstart:cstart + YCHUNK],
                              yt[:, cstart:cstart + YCHUNK])

        Ex = mybir.ActivationFunctionType.Exp
        Sq = mybir.ActivationFunctionType.Square

        # Trigger ACT function-table loads and absorb the one-time const-DMA
        # dependencies while no other deps are pending (hardware activation
        # instructions support a single sync wait).
        warm = const.tile([128, 1], f32)
        nc.scalar.activation(warm[:], bias_sb[:], Sq)
        nc.scalar.activation(warm[:], bias_sb[:], Ex, bias=bias_sb[:])

        zacc = None
        for t in range(NSPANS):
            ys, phis = [], []
            for half in range(2):
                h = 2 * t + half
                y_t = y_all[:, h * HALF:(h + 1) * HALF]
                ys.append(y_t)
                v = vpool.tile([128, HALF], f32)
                nc.tensor.matmul(v[:], a2_sb[:], y_t, start=True, stop=True)
                phi = phipool.tile([128, HALF], bf16)
                nc.scalar.activation(phi[:], v[:], Sq)
                phis.append(phi)
            d = dpool.tile([128, SPAN], f32)
            for half in range(2):
                sl = d[:, half * HALF:(half + 1) * HALF]
                nc.tensor.matmul(sl, gt_sb[:], phis[half][:],
                                 start=True, stop=False)
                nc.tensor.matmul(sl, c2w_sb[:], ys[half], start=False, stop=True)
            u = upool.tile([128, SPAN], bf16)
            nc.scalar.activation(u[:], d[:], Ex, bias=bias_sb[:])
            for half in range(2):
                h = 2 * t + half
                k = h % 4
                if k == 0:
                    zacc = zpool.tile([128, HALF], f32)
                nc.tensor.matmul(zacc[32 * k:32 * k + 1, :], ones_sb[:, 0:1],
                                 u[:, half * HALF:(half + 1) * HALF],
                                 start=True, stop=True,
                                 tile_position=(0, 32 * k))
                if k == 3 or h == NH - 1:
                    zst = lnzpool.tile([128, HALF], f32)
                    nc.vector.tensor_copy(zst[:], zacc[:])
                    nc.gpsimd.dma_start(zout[h // 4, :, :], zst[:])
    return nc


def _split_multi_waits(raw):
    """TRN2 instructions carry a single sync-wait slot; Tile emits several.
    Hoist all but the last wait of each instruction into single-wait NoOps
    on the same engine immediately before it (semantically identical)."""
    import json as _json
    bj = _json.loads(raw)
    n = 0
    for fn in bj["functions"]:
        for bb in fn["blocks"]:
            out = []
            for inst in bb["instructions"]:
                si = inst.get("sync_info")
                if si:
                    w = si.get("on_wait") or []
                    if len(w) > 1:
                        for ww in w[:-1]:
                            n += 1
                            out.append({
                                "debug": inst.get("debug", 0),
                                "engine": inst["engine"],
                                "ins": [], "outs": [],
                                "name": "I-ws%d" % n, "opcode": "NoOp",
                                "sync_info": {"on_wait": [ww],
                                              "on_update": []},
                            })
                        si["on_wait"] = [w[-1]]
                out.append(inst)
            bb["instructions"] = out
    return _json.dumps(bj).encode()


def _get_program():
    if "nc" not in _CACHE:
        nc = _build_bass()
        patched = _split_multi_waits(nc.to_json_bytes())
        nc.to_json_bytes = lambda: patched
        _CACHE["nc"] = nc
    return _CACHE["nc"]


def _run_device(YT_cores, gt_w, a2_w, c2w_w, bias_w):
    from concourse.bass_utils import run_bass_kernel_spmd
    nc = _get_program()
    in_maps = []
    for c in range(N_CORES):
        in_maps.append({
            "yt": YT_cores[c],
            "gt": gt_w, "a2": a2_w, "c2w": c2w_w, "bias": bias_w,
        })
    res = run_bass_kernel_spmd(nc, in_maps, list(range(N_CORES)))
    return [r["zout"] for r in res.results]


def kernel(points, centers, covs_inv_sqrt, weights, threshold):
    import ml_dtypes
    import time as _time

    pts = np.asarray(points, np.float32)
    S, T, lvec, kconst = _prep_model(centers, covs_inv_sqrt, weights)
    y32 = pts - np.float32(0.5)                                # [N,D] f32

    rng = np.random.default_rng(12345)
    coef = _fit_shift(y32, T, lvec, kconst, rng)               # [D+2] f64
    alpha, beta, gamma = coef[0], coef[1:1 + D], coef[1 + D]

    # Fold the shift into the model
    Tpp = T - alpha * np.eye(D)[None, :, :]                    # [M,D,D]
    lpp = lvec - beta[None, :]                                 # [M,D]
    kpp = kconst - gamma                                       # [M]

    A = _build_frame(rng)                                      # [128,D] (bf16 vals)
    G = _solve_frame_weights(A, Tpp)                           # [M,128]

    # Device weight tensors
    gt_w = _bf16(G.T)                                          # [128 r,128 j]
    a2_w = _bf16(np.concatenate([A.T, A.T], axis=0))           # [32,128]
    c2w_w = _bf16(np.concatenate([lpp.T, lpp.T], axis=0))      # [32,128]
    bias_w = np.asarray(kpp, np.float32).reshape(M, 1)

    # Per-core inputs: y hi/lo split, transposed + tiled [NH, 32, HALF]
    y_hi = y32.astype(ml_dtypes.bfloat16)
    y_lo = (y32 - y_hi.astype(np.float32)).astype(ml_dtypes.bfloat16)
    YT_cores = []
    for c in range(N_CORES):
        lo_i, hi_i = c * NPC, min((c + 1) * NPC, N)
        npts = hi_i - lo_i
        blk = np.zeros((NPAD, 32), dtype=ml_dtypes.bfloat16)
        blk[:npts, :16] = y_hi[lo_i:hi_i]
        blk[:npts, 16:] = y_lo[lo_i:hi_i]
        YT_cores.append(np.ascontiguousarray(blk.T))           # [32, NPAD]

    t0 = _time.time()
    zl = _run_device(YT_cores, gt_w, a2_w, c2w_w, bias_w)
    global LAST_EXEC_NS
    LAST_EXEC_NS = int((_time.time() - t0) * 1e9)

    zs = []
    for c, zc in enumerate(zl):
        zc = np.asarray(zc, np.float32)                        # [NH/4,128,HALF]
        zflat = zc[:, ::32, :].reshape(NPAD)                   # rows 0,32,64,96
        zs.append(zflat[:min((c + 1) * NPC, N) - c * NPC])
    z = np.concatenate(zs)                                     # [N]

    # Host: out = s + ln z - threshold
    ynorm = np.einsum('nd,nd->n', y32, y32, dtype=np.float64)
    s = alpha * ynorm + y32.astype(np.float64) @ beta + gamma  # [N]
    with np.errstate(divide='ignore', invalid='ignore', over='ignore'):
        out = s + np.log(z.astype(np.float64))

    # Exact fixup for points whose z left the fp32 window (rare)
    bad = ~np.isfinite(out) | (z <= 1e-30) | (z >= 1e30)
    if bad.any():
        yb = y32[bad].astype(np.float64)
        db = _d_exact(yb, T, lvec, kconst)
        mb = db.max(1, keepdims=True)
        out[bad] = (mb + np.log(np.exp(db - mb).sum(1, keepdims=True)))[:, 0]

    thr = np.asarray(threshold, np.float32).astype(np.float64)
    return (out[:, None] - thr[None, :]).astype(np.float32)


LAST_EXEC_NS = 0


# revision 27
# speedup vs baseline: 3.5349x; 3.5349x over previous
import numpy as np

# nn_GaussianMixture: log-likelihood of N points under an M-component GMM.
# Shapes hardcoded per contract: points [500000,16], centers [128,16],
# covs_inv_sqrt [128,16,16], weights [128], threshold [1].
#
# Device strategy (8 NeuronCores, data-parallel over points):
#   d_ij = -0.5 (x_i-c_j)^T S_j (x_i-c_j) + logcoef_j, out_i = logsumexp_j d_ij
# Rewritten in centered coords y = x - 0.5 as
#   d_ij = y^T T_j y + l_j^T y + k_j
# A per-point affine shift s(y) = alpha*|y|^2 + beta^T y + gamma (fit at
# runtime to max_j d) is folded into the weights so a single exp pass is
# numerically safe:  d'_ij = d_ij - s(y_i),  out = s + log sum_j exp(d').
# The quadratic y^T (T_j - alpha I) y is evaluated through a rank-1 frame:
#   (T_j - alpha I) ~= sum_r G_jr a_r a_r^T  =>  y^T T'' y = sum_r G_jr (a_r.y)^2
# so the device pipeline per point-tile is pure matmul + square + matmul:
#   stage1 (PE):  v_r = a_r . y              [K=32 matmul on y_hi/y_lo bf16]
#   square (ACT/DVE): w_r = v_r^2            -> bf16
#   main   (PE):  d' = G^T w + L^T y + bias  [K=128 + K=32, accumulated in PSUM]
#   exp    (ACT): u = exp(d' + bias_j)       [PSUM -> SBUF bf16]
#   zsum   (PE):  z_i = sum_j u_ij           [ones matmul]
# Host: out = s + ln z - threshold; rare out-of-range points recomputed exactly.

N, M, D = 500000, 128, 16
N_CORES = 8
HALF = 512                      # half-span (one PSUM bank of fp32)
SPAN = 2 * HALF
NPC = (N + N_CORES - 1) // N_CORES          # 62500 points per core
NSPANS = (NPC + SPAN - 1) // SPAN           # 62
NH = 2 * NSPANS                              # 124 half-spans
NPAD = NSPANS * SPAN                         # 63488

_CACHE = {}


def _bf16(a):
    import ml_dtypes
    return np.asarray(a, np.float32).astype(ml_dtypes.bfloat16)


def _prep_model(centers, covs_inv_sqrt, weights):
    """Exact f64 model constants in centered coordinates y = x - 0.5."""
    c = np.asarray(centers, np.float64)
    Lm = np.asarray(covs_inv_sqrt, np.float64)
    S = np.einsum('jde,jfe->jdf', Lm, Lm)                    # [M,D,D]
    w = np.abs(np.asarray(weights, np.float64))
    cprs = w / (w.sum() + 1e-30)
    sign, logdet = np.linalg.slogdet(S)
    lc = np.log(cprs + 1e-300) + 0.5 * logdet                # [M]
    ct = c - 0.5                                             # centered centers
    T = -0.5 * S                                             # quad coeff
    lvec = np.einsum('jde,je->jd', S, ct)                    # [M,D]
    kconst = -0.5 * np.einsum('jd,jd->j', ct, lvec * 0 + np.einsum('jde,je->jd', S, ct)) \
        if False else -0.5 * np.einsum('jd,jde,je->j', ct, S, ct)
    kconst = kconst + lc                                     # [M]
    return S, T, lvec, kconst


def _d_exact(y, T, lvec, kconst):
    """Exact d matrix for a batch of centered points y [n,D] (f64)."""
    q = np.einsum('nd,jde,ne->nj', y, T, y, optimize=True)
    return q + y @ lvec.T + kconst[None, :]


def _fit_shift(y, T, lvec, kconst, rng):
    """Affine s(y)=alpha*|y|^2+beta.y+gamma with max_j d - s centered in a
    safe exp window."""
    n = y.shape[0]
    idx = rng.choice(n, size=min(3072, n), replace=False)
    ys = y[idx].astype(np.float64)
    dm = _d_exact(ys, T, lvec, kconst)
    mx = dm.max(1)
    F = np.concatenate([(ys * ys).sum(1, keepdims=True), ys,
                        np.ones((len(ys), 1))], axis=1)
    coef, *_ = np.linalg.lstsq(F, mx, rcond=None)
    r = mx - F @ coef
    coef[-1] += (r.min() + r.max()) / 2.0                    # center the window
    return coef                                              # [D+2]


def _build_frame(rng):
    """128 rank-1 directions: 16 coordinate axes + 112 random unit vectors,
    quantized to bf16 (the solve below uses the quantized values exactly)."""
    A = np.zeros((128, D))
    A[:16] = np.eye(D)
    V = rng.standard_normal((112, D))
    V /= np.linalg.norm(V, axis=1, keepdims=True)
    A[16:] = V
    return _bf16(A).astype(np.float64)


def _solve_frame_weights(A, Ttgt):
    """G [M,128] minimizing ||sum_r G_jr a_r a_r^T - Ttgt_j||_F."""
    iu0, iu1 = np.triu_indices(D)
    wgt = np.where(iu0 == iu1, 1.0, np.sqrt(2.0))
    B = (np.einsum('rd,re->rde', A, A)[:, iu0, iu1] * wgt)   # [128,136]
    Tv = Ttgt[:, iu0, iu1] * wgt                             # [M,136]
    G = Tv @ np.linalg.pinv(B)                               # [M,128]
    return G


def _build_bass():
    import concourse.bass as bass
    import concourse.mybir as mybir
    from concourse import tile
    from contextlib import ExitStack

    bf16 = mybir.dt.bfloat16
    f32 = mybir.dt.float32
    nc = bass.Bass()

    yt = nc.dram_tensor("yt", [32, NPAD], bf16, kind="ExternalInput")
    gt = nc.dram_tensor("gt", [128, 128], bf16, kind="ExternalInput")
    a2 = nc.dram_tensor("a2", [32, 128], bf16, kind="ExternalInput")
    c2w = nc.dram_tensor("c2w", [32, 128], bf16, kind="ExternalInput")
    bias = nc.dram_tensor("bias", [128, 1], f32, kind="ExternalInput")
    zout = nc.dram_tensor("zout", [NH, HALF], f32, kind="ExternalOutput")

    with tile.TileContext(nc) as tc, ExitStack() as ctx:
        const = ctx.enter_context(tc.tile_pool(name="const", bufs=1))
        phipool = ctx.enter_context(tc.tile_pool(name="phipool", bufs=4))
        upool = ctx.enter_context(tc.tile_pool(name="upool", bufs=3))
        lnzpool = ctx.enter_context(tc.tile_pool(name="lnzpool", bufs=2))
        vpool = ctx.enter_context(tc.tile_pool(name="vpool", bufs=2, space="PSUM"))
        dpool = ctx.enter_context(tc.tile_pool(name="dpool", bufs=2, space="PSUM"))
        zpool = ctx.enter_context(tc.tile_pool(name="zpool", bufs=2, space="PSUM"))

        gt_sb = const.tile([128, 128], bf16)
        a2_sb = const.tile([32, 128], bf16)
        c2w_sb = const.tile([32, 128], bf16)
        bias_sb = const.tile([128, 1], f32)
        ones_sb = const.tile([128, 1], bf16)
        nc.sync.dma_start(gt_sb[:], gt[:, :])
        nc.sync.dma_start(a2_sb[:], a2[:, :])
        nc.sync.dma_start(c2w_sb[:], c2w[:, :])
        nc.sync.dma_start(bias_sb[:], bias[:, :])
        nc.vector.memset(ones_sb[:], 1.0)

        # Preload all point data: one persistent SBUF region, chunked
        # first-write DMAs (no pool rotation -> no WAW sync waits).
        y_all = const.tile([32, NPAD], bf16)
        YCHUNK = 4 * HALF
        for cstart in range(0, NPAD, YCHUNK):
            nc.sync.dma_start(y_all[:, cstart:cstart + YCHUNK],
                              yt[:, cstart:cstart + YCHUNK])

        Ex = mybir.ActivationFunctionType.Exp
        Sq = mybir.ActivationFunctionType.Square

        # Trigger ACT function-table loads and absorb the one-time const-DMA
        # dependencies while no other deps are pending (hardware activation
        # instructions support a single sync wait).
        warm = const.tile([128, 1], f32)
        nc.scalar.activation(warm[:], bias_sb[:], Sq)
        nc.scalar.activation(warm[:], bias_sb[:], Ex, bias=bias_sb[:])

        zacc = None
        for t in range(NSPANS):
            ys, phis = [], []
            for half in range(2):
                h = 2 * t + half
                y_t = y_all[:, h * HALF:(h + 1) * HALF]
                ys.append(y_t)
                v = vpool.tile([128, HALF], f32)
                nc.tensor.matmul(v[:], a2_sb[:], y_t, start=True, stop=True)
                phi = phipool.tile([128, HALF], bf16)
                nc.scalar.activation(phi[:], v[:], Sq)
                phis.append(phi)
            d = dpool.tile([128, SPAN], f32)
            for half in range(2):
                sl = d[:, half * HALF:(half + 1) * HALF]
                nc.tensor.matmul(sl, gt_sb[:], phis[half][:],
                                 start=True, stop=False)
                nc.tensor.matmul(sl, c2w_sb[:], ys[half], start=False, stop=True)
            u = upool.tile([128, SPAN], bf16)
            nc.scalar.activation(u[:], d[:], Ex, bias=bias_sb[:])
            for half in range(2):
                h = 2 * t + half
                k = h % 4
                if k == 0:
                    zacc = zpool.tile([128, HALF], f32)
                nc.tensor.matmul(zacc[32 * k:32 * k + 1, :], ones_sb[:, 0:1],
                                 u[:, half * HALF:(half + 1) * HALF],
                                 start=True, stop=True,
                                 tile_position=(0, 32 * k))
                if k == 3 or h == NH - 1:
                    g0 = h - k
                    zst = lnzpool.tile([128, HALF], f32)
                    nc.vector.tensor_copy(zst[:], zacc[:])
                    for kk in range(k + 1):
                        nc.sync.dma_start(zout[g0 + kk:g0 + kk + 1, :],
                                          zst[32 * kk:32 * kk + 1, :])
    return nc


def _split_multi_waits(raw):
    """TRN2 instructions carry a single sync-wait slot; Tile emits several.
    Hoist all but the last wait of each instruction into single-wait NoOps
    on the same engine immediately before it (semantically identical)."""
    import json as _json
    bj = _json.loads(raw)
    n = 0
    for fn in bj["functions"]:
        for bb in fn["blocks"]:
            out = []
            for inst in bb["instructions"]:
                si = inst.get("sync_info")
                if si:
                    w = si.get("on_wait") or []
                    if len(w) > 1:
                        for ww in w[:-1]:
                            n += 1
                            out.append({
                                "debug": inst.get("debug", 0),
                                "engine": inst["engine"],
                                "ins": [], "outs": [],
                                "name": "I-ws%d" % n, "opcode": "NoOp",
                                "sync_info": {"on_wait": [ww],
                                              "on_update": []},
                            })
                        si["on_wait"] = [w[-1]]
                out.append(inst)
            bb["instructions"] = out
    return _json.dumps(bj).encode()


def _get_program():
    if "nc" not in _CACHE:
        nc = _build_bass()
        patched = _split_multi_waits(nc.to_json_bytes())
        nc.to_json_bytes = lambda: patched
        _CACHE["nc"] = nc
    return _CACHE["nc"]


def _run_device(YT_cores, gt_w, a2_w, c2w_w, bias_w):
    import os
    from concourse.bass_utils import run_bass_kernel_spmd
    nc = _get_program()
    in_maps = []
    for c in range(N_CORES):
        in_maps.append({
            "yt": YT_cores[c],
            "gt": gt_w, "a2": a2_w, "c2w": c2w_w, "bias": bias_w,
        })
    trace = os.environ.get("GMM_TRACE") == "1"
    res = run_bass_kernel_spmd(nc, in_maps, list(range(N_CORES)), trace=trace)
    global LAST_HW_NS, LAST_PROFILE
    if getattr(res, "exec_time_ns", None):
        LAST_HW_NS = res.exec_time_ns
    LAST_PROFILE = getattr(res, "profile_json", None)
    return [r["zout"] for r in res.results]


def kernel(points, centers, covs_inv_sqrt, weights, threshold):
    import ml_dtypes
    import time as _time

    pts = np.asarray(points, np.float32)
    S, T, lvec, kconst = _prep_model(centers, covs_inv_sqrt, weights)
    y32 = pts - np.float32(0.5)                                # [N,D] f32

    rng = np.random.default_rng(12345)
    coef = _fit_shift(y32, T, lvec, kconst, rng)               # [D+2] f64
    alpha, beta, gamma = coef[0], coef[1:1 + D], coef[1 + D]

    # Fold the shift into the model
    Tpp = T - alpha * np.eye(D)[None, :, :]                    # [M,D,D]
    lpp = lvec - beta[None, :]                                 # [M,D]
    kpp = kconst - gamma                                       # [M]

    A = _build_frame(rng)                                      # [128,D] (bf16 vals)
    G = _solve_frame_weights(A, Tpp)                           # [M,128]

    # Device weight tensors
    gt_w = _bf16(G.T)                                          # [128 r,128 j]
    a2_w = _bf16(np.concatenate([A.T, A.T], axis=0))           # [32,128]
    c2w_w = _bf16(np.concatenate([lpp.T, lpp.T], axis=0))      # [32,128]
    bias_w = np.asarray(kpp, np.float32).reshape(M, 1)

    # Per-core inputs: y hi/lo split, transposed + tiled [NH, 32, HALF]
    y_hi = y32.astype(ml_dtypes.bfloat16)
    y_lo = (y32 - y_hi.astype(np.float32)).astype(ml_dtypes.bfloat16)
    YT_cores = []
    for c in range(N_CORES):
        lo_i, hi_i = c * NPC, min((c + 1) * NPC, N)
        npts = hi_i - lo_i
        blk = np.zeros((NPAD, 32), dtype=ml_dtypes.bfloat16)
        blk[:npts, :16] = y_hi[lo_i:hi_i]
        blk[:npts, 16:] = y_lo[lo_i:hi_i]
        YT_cores.append(np.ascontiguousarray(blk.T))           # [32, NPAD]

    t0 = _time.time()
    zl = _run_device(YT_cores, gt_w, a2_w, c2w_w, bias_w)
    global LAST_EXEC_NS
    LAST_EXEC_NS = int((_time.time() - t0) * 1e9)

    zs = []
    for c, zc in enumerate(zl):
        zflat = np.asarray(zc, np.float32).reshape(NPAD)       # [NH,HALF]
        zs.append(zflat[:min((c + 1) * NPC, N) - c * NPC])
    z = np.concatenate(zs)                                     # [N]

    # Host: out = s + ln z - threshold
    ynorm = np.einsum('nd,nd->n', y32, y32, dtype=np.float64)
    s = alpha * ynorm + y32.astype(np.float64) @ beta + gamma  # [N]
    with np.errstate(divide='ignore', invalid='ignore', over='ignore'):
        out = s + np.log(z.astype(np.float64))

    # Exact fixup for points whose z left the fp32 window (rare)
    bad = ~np.isfinite(out) | (z <= 1e-30) | (z >= 1e30)
    if bad.any():
        yb = y32[bad].astype(np.float64)
        db = _d_exact(yb, T, lvec, kconst)
        mb = db.max(1, keepdims=True)
        out[bad] = (mb + np.log(np.exp(db - mb).sum(1, keepdims=True)))[:, 0]

    thr = np.asarray(threshold, np.float32).astype(np.float64)
    return (out[:, None] - thr[None, :]).astype(np.float32)


LAST_EXEC_NS = 0
LAST_HW_NS = 0
LAST_PROFILE = None


# revision 32
# speedup vs baseline: 4.2414x; 1.1999x over previous
import numpy as np

# nn_GaussianMixture: log-likelihood of N points under an M-component GMM.
# Shapes hardcoded per contract: points [500000,16], centers [128,16],
# covs_inv_sqrt [128,16,16], weights [128], threshold [1].
#
# Device strategy (8 NeuronCores, data-parallel over points):
#   d_ij = -0.5 (x_i-c_j)^T S_j (x_i-c_j) + logcoef_j, out_i = logsumexp_j d_ij
# Rewritten in centered coords y = x - 0.5 as
#   d_ij = y^T T_j y + l_j^T y + k_j
# A per-point affine shift s(y) = alpha*|y|^2 + beta^T y + gamma (fit at
# runtime to max_j d) is folded into the weights so a single exp pass is
# numerically safe:  d'_ij = d_ij - s(y_i),  out = s + log sum_j exp(d').
# The quadratic y^T (T_j - alpha I) y is evaluated through a rank-1 frame:
#   (T_j - alpha I) ~= sum_r G_jr a_r a_r^T  =>  y^T T'' y = sum_r G_jr (a_r.y)^2
# so the device pipeline per point-tile is pure matmul + square + matmul:
#   stage1 (PE):  v_r = a_r . y              [K=32 matmul on y_hi/y_lo bf16]
#   square (ACT/DVE): w_r = v_r^2            -> bf16
#   main   (PE):  d' = G^T w + L^T y + bias  [K=128 + K=32, accumulated in PSUM]
#   exp    (ACT): u = exp(d' + bias_j)       [PSUM -> SBUF bf16]
#   zsum   (PE):  z_i = sum_j u_ij           [ones matmul]
# Host: out = s + ln z - threshold; rare out-of-range points recomputed exactly.

N, M, D = 500000, 128, 16
N_CORES = 8
HALF = 512                      # half-span (one PSUM bank of fp32)
SPAN = 2 * HALF
NPC = (N + N_CORES - 1) // N_CORES          # 62500 points per core
NSPANS = (NPC + SPAN - 1) // SPAN           # 62
NH = 2 * NSPANS                              # 124 half-spans
NPAD = NSPANS * SPAN                         # 63488

_CACHE = {}


def _bf16(a):
    import ml_dtypes
    return np.asarray(a, np.float32).astype(ml_dtypes.bfloat16)


def _prep_model(centers, covs_inv_sqrt, weights):
    """Exact f64 model constants in centered coordinates y = x - 0.5."""
    c = np.asarray(centers, np.float64)
    Lm = np.asarray(covs_inv_sqrt, np.float64)
    S = np.einsum('jde,jfe->jdf', Lm, Lm)                    # [M,D,D]
    w = np.abs(np.asarray(weights, np.float64))
    cprs = w / (w.sum() + 1e-30)
    sign, logdet = np.linalg.slogdet(S)
    lc = np.log(cprs + 1e-300) + 0.5 * logdet                # [M]
    ct = c - 0.5                                             # centered centers
    T = -0.5 * S                                             # quad coeff
    lvec = np.einsum('jde,je->jd', S, ct)                    # [M,D]
    kconst = -0.5 * np.einsum('jd,jd->j', ct, lvec * 0 + np.einsum('jde,je->jd', S, ct)) \
        if False else -0.5 * np.einsum('jd,jde,je->j', ct, S, ct)
    kconst = kconst + lc                                     # [M]
    return S, T, lvec, kconst


def _d_exact(y, T, lvec, kconst):
    """Exact d matrix for a batch of centered points y [n,D] (f64)."""
    q = np.einsum('nd,jde,ne->nj', y, T, y, optimize=True)
    return q + y @ lvec.T + kconst[None, :]


def _fit_shift(y, T, lvec, kconst, rng):
    """Affine s(y)=alpha*|y|^2+beta.y+gamma with max_j d - s centered in a
    safe exp window."""
    n = y.shape[0]
    idx = rng.choice(n, size=min(3072, n), replace=False)
    ys = y[idx].astype(np.float64)
    dm = _d_exact(ys, T, lvec, kconst)
    mx = dm.max(1)
    F = np.concatenate([(ys * ys).sum(1, keepdims=True), ys,
                        np.ones((len(ys), 1))], axis=1)
    coef, *_ = np.linalg.lstsq(F, mx, rcond=None)
    r = mx - F @ coef
    coef[-1] += (r.min() + r.max()) / 2.0                    # center the window
    return coef                                              # [D+2]


def _build_frame(rng):
    """128 rank-1 directions: 16 coordinate axes + 112 random unit vectors,
    quantized to bf16 (the solve below uses the quantized values exactly)."""
    A = np.zeros((128, D))
    A[:16] = np.eye(D)
    V = rng.standard_normal((112, D))
    V /= np.linalg.norm(V, axis=1, keepdims=True)
    A[16:] = V
    return _bf16(A).astype(np.float64)


def _solve_frame_weights(A, Ttgt):
    """G [M,128] minimizing ||sum_r G_jr a_r a_r^T - Ttgt_j||_F."""
    iu0, iu1 = np.triu_indices(D)
    wgt = np.where(iu0 == iu1, 1.0, np.sqrt(2.0))
    B = (np.einsum('rd,re->rde', A, A)[:, iu0, iu1] * wgt)   # [128,136]
    Tv = Ttgt[:, iu0, iu1] * wgt                             # [M,136]
    G = Tv @ np.linalg.pinv(B)                               # [M,128]
    return G


def _build_bass():
    import concourse.bass as bass
    import concourse.mybir as mybir
    from concourse import tile
    from contextlib import ExitStack

    bf16 = mybir.dt.bfloat16
    f32 = mybir.dt.float32
    nc = bass.Bass()

    yt = nc.dram_tensor("yt", [32, NPAD], bf16, kind="ExternalInput")
    gt = nc.dram_tensor("gt", [128, 128], bf16, kind="ExternalInput")
    a2 = nc.dram_tensor("a2", [32, 128], bf16, kind="ExternalInput")
    c2w = nc.dram_tensor("c2w", [32, 128], bf16, kind="ExternalInput")
    bias = nc.dram_tensor("bias", [128, 1], f32, kind="ExternalInput")
    zout = nc.dram_tensor("zout", [NH, HALF], f32, kind="ExternalOutput")

    with tile.TileContext(nc) as tc, ExitStack() as ctx:
        const = ctx.enter_context(tc.tile_pool(name="const", bufs=1))
        vsbpool = ctx.enter_context(tc.tile_pool(name="vsbpool", bufs=3))
        phipool = ctx.enter_context(tc.tile_pool(name="phipool", bufs=3))
        upool = ctx.enter_context(tc.tile_pool(name="upool", bufs=3))
        zsbpool = ctx.enter_context(tc.tile_pool(name="zsbpool", bufs=3))
        vpool = ctx.enter_context(tc.tile_pool(name="vpool", bufs=2, space="PSUM"))
        dpool = ctx.enter_context(tc.tile_pool(name="dpool", bufs=2, space="PSUM"))

        gt_sb = const.tile([128, 128], bf16)
        a2_sb = const.tile([32, 128], bf16)
        c2w_sb = const.tile([32, 128], bf16)
        bias_sb = const.tile([128, 1], f32)
        ones_sb = const.tile([128, 32], bf16)
        nc.sync.dma_start(gt_sb[:], gt[:, :])
        nc.sync.dma_start(a2_sb[:], a2[:, :])
        nc.sync.dma_start(c2w_sb[:], c2w[:, :])
        nc.sync.dma_start(bias_sb[:], bias[:, :])
        nc.vector.memset(ones_sb[:], 1.0)

        # Preload all point data: one persistent SBUF region, chunked
        # first-write DMAs (no pool rotation -> no WAW sync waits).
        y_all = const.tile([32, NPAD], bf16)
        YCHUNK = 4 * HALF
        for cstart in range(0, NPAD, YCHUNK):
            nc.sync.dma_start(y_all[:, cstart:cstart + YCHUNK],
                              yt[:, cstart:cstart + YCHUNK])

        Ex = mybir.ActivationFunctionType.Exp

        # Absorb the one-time const-DMA dependency and ACT table load while
        # no other deps are pending (activation instructions carry a single
        # sync wait).
        warm = const.tile([128, 1], f32)
        nc.scalar.activation(warm[:], bias_sb[:], Ex, bias=bias_sb[:])

        for t in range(NSPANS):
            y0 = y_all[:, (2 * t) * HALF:(2 * t + 1) * HALF]
            y1 = y_all[:, (2 * t + 1) * HALF:(2 * t + 2) * HALF]
            v = vpool.tile([128, SPAN], f32)
            nc.tensor.matmul(v[:, 0:HALF], a2_sb[:], y0, start=True, stop=True)
            nc.tensor.matmul(v[:, HALF:SPAN], a2_sb[:], y1, start=True, stop=True)
            vsb = vsbpool.tile([128, SPAN], bf16)
            nc.vector.tensor_copy(vsb[:], v[:])
            phi = phipool.tile([128, SPAN], bf16)
            nc.gpsimd.tensor_mul(phi[:], vsb[:], vsb[:])
            d = dpool.tile([128, SPAN], f32)
            nc.tensor.matmul(d[:, 0:HALF], gt_sb[:], phi[:, 0:HALF],
                             start=True, stop=False)
            nc.tensor.matmul(d[:, HALF:SPAN], gt_sb[:], phi[:, HALF:SPAN],
                             start=True, stop=False)
            nc.tensor.matmul(d[:, 0:HALF], c2w_sb[:], y0, start=False, stop=True)
            nc.tensor.matmul(d[:, HALF:SPAN], c2w_sb[:], y1, start=False, stop=True)
            u = upool.tile([128, SPAN], bf16)
            nc.scalar.activation(u[:], d[:], Ex, bias=bias_sb[:])
            # z sums overwrite rows 0-63 of the d tile (read by exp already)
            nc.tensor.matmul(d[0:32, 0:HALF], ones_sb[:], u[:, 0:HALF],
                             start=True, stop=True, tile_position=(0, 0))
            nc.tensor.matmul(d[32:64, 0:HALF], ones_sb[:], u[:, HALF:SPAN],
                             start=True, stop=True, tile_position=(0, 32))
            zst = zsbpool.tile([64, HALF], f32)
            nc.vector.tensor_copy(zst[:], d[0:64, 0:HALF])
            nc.sync.dma_start(zout[2 * t:2 * t + 1, :], zst[0:1, :])
            nc.sync.dma_start(zout[2 * t + 1:2 * t + 2, :], zst[32:33, :])
    return nc


def _split_multi_waits(raw):
    """TRN2 instructions carry a single sync-wait slot; Tile emits several.
    Hoist all but the last wait of each instruction into single-wait NoOps
    on the same engine immediately before it (semantically identical)."""
    import json as _json
    bj = _json.loads(raw)
    n = 0
    for fn in bj["functions"]:
        for bb in fn["blocks"]:
            out = []
            for inst in bb["instructions"]:
                si = inst.get("sync_info")
                if si:
                    w = si.get("on_wait") or []
                    if len(w) > 1:
                        for ww in w[:-1]:
                            n += 1
                            out.append({
                                "debug": inst.get("debug", 0),
                                "engine": inst["engine"],
                                "ins": [], "outs": [],
                                "name": "I-ws%d" % n, "opcode": "NoOp",
                                "sync_info": {"on_wait": [ww],
                                              "on_update": []},
                            })
                        si["on_wait"] = [w[-1]]
                out.append(inst)
            bb["instructions"] = out
    return _json.dumps(bj).encode()


def _get_program():
    if "nc" not in _CACHE:
        nc = _build_bass()
        patched = _split_multi_waits(nc.to_json_bytes())
        nc.to_json_bytes = lambda: patched
        _CACHE["nc"] = nc
    return _CACHE["nc"]


def _get_runner():
    """Sharded pjrt runner mirroring bass2jax.run_bass_via_pjrt, but taking
    pre-sharded device-resident inputs so host->device transfer can overlap
    across cores (the axon tunnel is ~2x faster with concurrent puts)."""
    if "runner" in _CACHE:
        return _CACHE["runner"]
    import jax
    import concourse.mybir as mybir
    from jax.experimental.shard_map import shard_map
    from jax.sharding import Mesh, PartitionSpec, NamedSharding
    from concourse import bass2jax

    bass2jax.install_neuronx_cc_hook()
    nc = _get_program()
    partition_name = (nc.partition_id_tensor.name
                      if nc.partition_id_tensor else None)
    in_names, out_names, out_avals, zero_outs = [], [], [], []
    for alloc in nc.m.functions[0].allocations:
        if not isinstance(alloc, mybir.MemoryLocationSet):
            continue
        name = alloc.memorylocations[0].name
        if alloc.kind == "ExternalInput":
            if name != partition_name:
                in_names.append(name)
        elif alloc.kind == "ExternalOutput":
            out_names.append(name)
            shape = tuple(alloc.tensor_shape)
            dtype = mybir.dt.np(alloc.dtype)
            out_avals.append(jax.core.ShapedArray(shape, dtype))
            zero_outs.append(np.zeros(shape, dtype))
    n_params = len(in_names)
    all_names = in_names + out_names

    def _body(*args):
        operands = list(args)
        if partition_name is not None:
            operands.append(bass2jax.partition_id_tensor())
        outs = bass2jax._bass_exec_p.bind(
            *operands,
            out_avals=tuple(out_avals),
            in_names=tuple(all_names + ([partition_name] if partition_name else [])),
            out_names=tuple(out_names),
            lowering_input_output_aliases=(),
            sim_require_finite=True,
            sim_require_nnan=True,
            nc=nc,
        )
        return tuple(outs)

    devices = jax.devices()[:N_CORES]
    mesh = Mesh(np.asarray(devices), ("core",))
    spec = PartitionSpec("core")
    sharding = NamedSharding(mesh, spec)
    donate = tuple(range(n_params, n_params + len(out_names)))
    fn = jax.jit(
        shard_map(_body, mesh=mesh, in_specs=(spec,) * len(all_names),
                  out_specs=(spec,) * len(out_names), check_rep=False),
        donate_argnums=donate, keep_unused=True,
    )
    _CACHE["runner"] = (fn, in_names, out_names, zero_outs, devices, sharding)
    return _CACHE["runner"]


def _put_sharded(global_np, devices, sharding):
    """device_put per-core slices concurrently, assemble one sharded array."""
    import jax
    import concurrent.futures as cf
    n = len(devices)
    rows = global_np.shape[0] // n

    def put(c):
        return jax.device_put(global_np[c * rows:(c + 1) * rows], devices[c])
    with cf.ThreadPoolExecutor(n) as ex:
        shards = list(ex.map(put, range(n)))
    return jax.make_array_from_single_device_arrays(
        global_np.shape, sharding, shards)


def _run_device(YT_global, gt_w, a2_w, c2w_w, bias_w):
    import jax
    fn, in_names, out_names, zero_outs, devices, sharding = _get_runner()
    per_input = {
        "yt": YT_global,
        "gt": np.concatenate([gt_w] * N_CORES, 0),
        "a2": np.concatenate([a2_w] * N_CORES, 0),
        "c2w": np.concatenate([c2w_w] * N_CORES, 0),
        "bias": np.concatenate([bias_w] * N_CORES, 0),
    }
    args = [_put_sharded(per_input[nm], devices, sharding) for nm in in_names]
    zouts = [_put_sharded(
        np.zeros((N_CORES * z.shape[0],) + z.shape[1:], z.dtype),
        devices, sharding) for z in zero_outs]
    out = fn(*args, *zouts)
    res = np.asarray(out[0])                                  # [8*NH, HALF]
    return res.reshape(N_CORES, NH, HALF)


def kernel(points, centers, covs_inv_sqrt, weights, threshold):
    import ml_dtypes
    import time as _time

    pts = np.asarray(points, np.float32)
    S, T, lvec, kconst = _prep_model(centers, covs_inv_sqrt, weights)
    y32 = pts - np.float32(0.5)                                # [N,D] f32

    rng = np.random.default_rng(12345)
    coef = _fit_shift(y32, T, lvec, kconst, rng)               # [D+2] f64
    alpha, beta, gamma = coef[0], coef[1:1 + D], coef[1 + D]

    # Fold the shift into the model
    Tpp = T - alpha * np.eye(D)[None, :, :]                    # [M,D,D]
    lpp = lvec - beta[None, :]                                 # [M,D]
    kpp = kconst - gamma                                       # [M]

    A = _build_frame(rng)                                      # [128,D] (bf16 vals)
    G = _solve_frame_weights(A, Tpp)                           # [M,128]

    # Device weight tensors
    gt_w = _bf16(G.T)                                          # [128 r,128 j]
    a2_w = _bf16(np.concatenate([A.T, A.T], axis=0))           # [32,128]
    c2w_w = _bf16(np.concatenate([lpp.T, lpp.T], axis=0))      # [32,128]
    bias_w = np.asarray(kpp, np.float32).reshape(M, 1)

    # Global sharded input: per-core [32, NPAD] blocks stacked -> [256, NPAD]
    y_hi = y32.astype(ml_dtypes.bfloat16)
    y_lo = (y32 - y_hi.astype(np.float32)).astype(ml_dtypes.bfloat16)
    ypacked = np.zeros((N_CORES * NPAD, 32), dtype=ml_dtypes.bfloat16)
    for c in range(N_CORES):
        lo_i, hi_i = c * NPC, min((c + 1) * NPC, N)
        ypacked[c * NPAD:c * NPAD + hi_i - lo_i, :16] = y_hi[lo_i:hi_i]
        ypacked[c * NPAD:c * NPAD + hi_i - lo_i, 16:] = y_lo[lo_i:hi_i]
    YT_global = np.ascontiguousarray(
        ypacked.reshape(N_CORES, NPAD, 32).transpose(0, 2, 1)
    ).reshape(N_CORES * 32, NPAD)

    t0 = _time.time()
    zarr = _run_device(YT_global, gt_w, a2_w, c2w_w, bias_w)   # [8, NH, HALF]
    global LAST_EXEC_NS
    LAST_EXEC_NS = int((_time.time() - t0) * 1e9)

    zs = []
    for c in range(N_CORES):
        zflat = np.asarray(zarr[c], np.float32).reshape(NPAD)
        zs.append(zflat[:min((c + 1) * NPC, N) - c * NPC])
    z = np.concatenate(zs)                                     # [N]

    # Host: out = s + ln z - threshold
    ynorm = np.einsum('nd,nd->n', y32, y32, dtype=np.float64)
    s = alpha * ynorm + y32.astype(np.float64) @ beta + gamma  # [N]
    with np.errstate(divide='ignore', invalid='ignore', over='ignore'):
        out = s + np.log(z.astype(np.float64))

    # Exact fixup for points whose z left the fp32 window (rare)
    bad = ~np.isfinite(out) | (z <= 1e-30) | (z >= 1e30)
    if bad.any():
        yb = y32[bad].astype(np.float64)
        db = _d_exact(yb, T, lvec, kconst)
        mb = db.max(1, keepdims=True)
        out[bad] = (mb + np.log(np.exp(db - mb).sum(1, keepdims=True)))[:, 0]

    thr = np.asarray(threshold, np.float32).astype(np.float64)
    return (out[:, None] - thr[None, :]).astype(np.float32)


LAST_EXEC_NS = 0
LAST_HW_NS = 0
LAST_PROFILE = None
